# revision 3
# baseline (speedup 1.0000x reference)
"""Trainium2 Bass kernel for LocationSensitiveAttention (sparse window attention).

The reference masks every position outside a 7-wide window centered near
prev_max_attentions[b] with -2^32 before the softmax, so alignments are
exactly zero outside that window.  The kernel exploits this: per core
(16 batch rows), it indirect-DMA-gathers only the 112 = 16*7 relevant
memory rows and conv state windows, computes the Bahdanau energy for
those lanes with a handful of small matmuls, does a 7-lane softmax +
argmax, and scatters the results into pre-zeroed outputs.  next_state is
a bulk DRAM->DRAM copy of state plus a 112-value scatter update.

Sharding: data-parallel over batch B=128 across 8 cores (16 rows each);
the small conv/dense/v_a parameters are replicated.
"""
import sys

sys.path.insert(0, "/opt/trn_rl_repo")

import numpy as np

B, T, U, F, K = 128, 4096, 128, 32, 31
WIN = 7
NCORES = 8
BL = B // NCORES           # 16 rows per core
P = BL * WIN               # 112 lanes per core
TP = T + 32                # padded state row pitch (15 front, 17 back)
SPAD_LEN = BL * TP + 64    # tail pad for 32-wide window gathers
NEG = np.float32(-2.0**32 + 1)
BIG = np.float32(1048576.0)  # 2^20: t - BIG + BIG is exact in f32

_CACHE = {}


def _build():
    from concourse import bass, mybir
    import concourse.tile as tile

    f32, i32 = mybir.dt.float32, mybir.dt.int32
    AF = mybir.ActivationFunctionType
    OP = mybir.AluOpType

    nc = bass.Bass()

    # inputs (per core)
    mem2d = nc.declare_dram_parameter("mem2d", [BL * T, U], f32, isOutput=False)
    spad = nc.declare_dram_parameter("spad", [SPAD_LEN], f32, isOutput=False)
    state_flat = nc.declare_dram_parameter("state_flat", [BL * T], f32, isOutput=False)
    q_in = nc.declare_dram_parameter("q_in", [BL, U], f32, isOutput=False)
    convw_in = nc.declare_dram_parameter("convw_in", [K, F], f32, isOutput=False)
    convb_in = nc.declare_dram_parameter("convb_in", [F, 1], f32, isOutput=False)
    locw_in = nc.declare_dram_parameter("locw_in", [F, U], f32, isOutput=False)
    ba_in = nc.declare_dram_parameter("ba_in", [1, U], f32, isOutput=False)
    va_in = nc.declare_dram_parameter("va_in", [U, 1], f32, isOutput=False)
    idx_in = nc.declare_dram_parameter("idx_in", [P, 3], i32, isOutput=False)  # win, mem, scat
    wmask_in = nc.declare_dram_parameter("wmask_in", [1, P], f32, isOutput=False)
    tvd_in = nc.declare_dram_parameter("tvd_in", [1, P], f32, isOutput=False)
    oneh_in = nc.declare_dram_parameter("oneh_in", [BL, P], f32, isOutput=False)
    ones_in = nc.declare_dram_parameter("ones_in", [1, P], f32, isOutput=False)
    ident_in = nc.declare_dram_parameter("ident_in", [P, P], f32, isOutput=False)

    # outputs (per core); align_out relies on PJRT zero-donated buffers
    align_out = nc.declare_dram_parameter("align_out", [BL * T], f32, isOutput=True)
    ns_out = nc.declare_dram_parameter("ns_out", [BL * T], f32, isOutput=True)
    ma_out = nc.declare_dram_parameter("ma_out", [BL], i32, isOutput=True)

    with tile.TileContext(nc) as tc:
        with tc.tile_pool(name="sb", bufs=1) as sb, \
             tc.tile_pool(name="ps", bufs=1, space="PSUM") as ps:
            # bulk next_state = state copy, issued first (no deps)
            nc.sync.dma_start(out=ns_out[:], in_=state_flat[:])

            # small loads
            idx = sb.tile([P, 3], i32)
            nc.gpsimd.dma_start(out=idx[:], in_=idx_in[:])
            q_sb = sb.tile([BL, U], f32)
            nc.sync.dma_start(out=q_sb[:], in_=q_in[:])
            cw_sb = sb.tile([K, F], f32)
            nc.sync.dma_start(out=cw_sb[:], in_=convw_in[:])
            cb_sb = sb.tile([F, 1], f32)
            nc.sync.dma_start(out=cb_sb[:], in_=convb_in[:])
            lw_sb = sb.tile([F, U], f32)
            nc.sync.dma_start(out=lw_sb[:], in_=locw_in[:])
            ba_sb = sb.tile([1, U], f32)
            nc.sync.dma_start(out=ba_sb[:], in_=ba_in[:])
            va_sb = sb.tile([U, 1], f32)
            nc.sync.dma_start(out=va_sb[:], in_=va_in[:])
            wm_sb = sb.tile([1, P], f32)
            nc.sync.dma_start(out=wm_sb[:], in_=wmask_in[:])
            tvd_sb = sb.tile([1, P], f32)
            nc.sync.dma_start(out=tvd_sb[:], in_=tvd_in[:])
            oh_sb = sb.tile([BL, P], f32)
            nc.sync.dma_start(out=oh_sb[:], in_=oneh_in[:])
            on_sb = sb.tile([1, P], f32)
            nc.sync.dma_start(out=on_sb[:], in_=ones_in[:])
            id_sb = sb.tile([P, P], f32)
            nc.sync.dma_start(out=id_sb[:], in_=ident_in[:])

            # gathers: conv windows W112[p, i] = spad[idx_win[p] + i]
            w_sb = sb.tile([P, 32], f32)
            nc.gpsimd.indirect_dma_start(
                out=w_sb[:], out_offset=None,
                in_=spad[:, None],
                in_offset=bass.IndirectOffsetOnAxis(ap=idx[:, 0:1], axis=0),
            )
            # memory rows mem_sb[p, :] = mem2d[idx_mem[p], :]
            mem_sb = sb.tile([P, U], f32)
            nc.gpsimd.indirect_dma_start(
                out=mem_sb[:], out_offset=None,
                in_=mem2d[:],
                in_offset=bass.IndirectOffsetOnAxis(ap=idx[:, 1:2], axis=0),
            )

            # conv: WT = W112^T  (matmul with identity)
            wt_ps = ps.tile([32, P], f32, space="PSUM")
            nc.tensor.matmul(out=wt_ps[:], lhsT=w_sb[:], rhs=id_sb[:],
                             start=True, stop=True)
            wt_sb = sb.tile([32, P], f32)
            nc.scalar.activation(out=wt_sb[:], in_=wt_ps[:], func=AF.Identity)
            # loc_feat^T [F, P] = conv_w^T @ WT + conv_b
            lf_ps = ps.tile([F, P], f32, space="PSUM")
            nc.tensor.matmul(out=lf_ps[:], lhsT=cw_sb[:], rhs=wt_sb[0:K, :],
                             start=True, stop=True)
            lf_sb = sb.tile([F, P], f32)
            nc.scalar.activation(out=lf_sb[:], in_=lf_ps[:], func=AF.Identity,
                                 bias=cb_sb[:, 0:1])

            # big accumulation [U, P]: mem^T + loc_w^T @ lf + query + b_a
            big_ps = ps.tile([U, P], f32, space="PSUM")
            nc.tensor.matmul(out=big_ps[:], lhsT=mem_sb[:], rhs=id_sb[:],
                             start=True, stop=False)
            nc.tensor.matmul(out=big_ps[:], lhsT=lw_sb[:], rhs=lf_sb[:],
                             start=False, stop=False)
            nc.tensor.matmul(out=big_ps[:], lhsT=q_sb[:], rhs=oh_sb[:],
                             start=False, stop=False)
            nc.tensor.matmul(out=big_ps[:], lhsT=ba_sb[:], rhs=on_sb[:],
                             start=False, stop=True)
            th_sb = sb.tile([U, P], f32)
            nc.scalar.activation(out=th_sb[:], in_=big_ps[:], func=AF.Tanh)

            # energy e[0, p] = sum_u v_a[u] * tanh[u, p]
            e_ps = ps.tile([1, P], f32, space="PSUM")
            nc.tensor.matmul(out=e_ps[:], lhsT=va_sb[:], rhs=th_sb[:],
                             start=True, stop=True)

            # masked 7-lane softmax in [1, 112] layout
            em = sb.tile([1, P], f32)
            nc.vector.tensor_tensor(out=em[:], in0=e_ps[:], in1=wm_sb[:], op=OP.add)
            em3 = em[:].rearrange("p (b j) -> p b j", j=WIN)
            rowmax = sb.tile([1, BL], f32)
            nc.vector.tensor_reduce(out=rowmax[:], in_=em3, axis=mybir.AxisListType.X,
                                    op=OP.max)
            rm_b = rowmax[:].unsqueeze(-1).to_broadcast([1, BL, WIN])
            es = sb.tile([1, P], f32)
            nc.vector.tensor_tensor(out=es[:].rearrange("p (b j) -> p b j", j=WIN),
                                    in0=em3, in1=rm_b, op=OP.subtract)
            ex = sb.tile([1, P], f32)
            nc.scalar.activation(out=ex[:], in_=es[:], func=AF.Exp)
            rowsum = sb.tile([1, BL], f32)
            nc.vector.tensor_reduce(out=rowsum[:], in_=ex[:].rearrange("p (b j) -> p b j", j=WIN),
                                    axis=mybir.AxisListType.X, op=OP.add)
            rc = sb.tile([1, BL], f32)
            nc.vector.reciprocal(out=rc[:], in_=rowsum[:])
            rc_b = rc[:].unsqueeze(-1).to_broadcast([1, BL, WIN])
            al1 = sb.tile([1, P], f32)
            nc.vector.tensor_tensor(out=al1[:].rearrange("p (b j) -> p b j", j=WIN),
                                    in0=ex[:].rearrange("p (b j) -> p b j", j=WIN),
                                    in1=rc_b, op=OP.mult)

            # argmax: lanes where ex == 1.0 are the (first) max
            eq = sb.tile([1, P], f32)
            nc.vector.tensor_scalar(out=eq[:], in0=ex[:], scalar1=1.0, scalar2=None,
                                    op0=OP.is_equal)
            cd = sb.tile([1, P], f32)
            nc.vector.tensor_tensor(out=cd[:], in0=eq[:], in1=tvd_sb[:], op=OP.mult)
            cd2 = sb.tile([1, P], f32)
            nc.vector.tensor_scalar(out=cd2[:], in0=cd[:], scalar1=float(BIG),
                                    scalar2=None, op0=OP.add)
            argt = sb.tile([1, BL], f32)
            nc.vector.tensor_reduce(out=argt[:], in_=cd2[:].rearrange("p (b j) -> p b j", j=WIN),
                                    axis=mybir.AxisListType.X, op=OP.min)
            mai = sb.tile([1, BL], i32)
            nc.vector.tensor_copy(out=mai[:], in_=argt[:])
            nc.sync.dma_start(out=ma_out[:], in_=mai[:])

            # transpose alignments to [112, 1] lanes for the scatters
            al_ps = ps.tile([P, 1], f32, space="PSUM")
            nc.tensor.matmul(out=al_ps[:], lhsT=al1[:], rhs=on_sb[:, 0:1],
                             start=True, stop=True)
            al112 = sb.tile([P, 1], f32)
            nc.vector.tensor_copy(out=al112[:], in_=al_ps[:])
            nsv = sb.tile([P, 1], f32)
            # state[b, t_p] is column 15 of the gathered conv window
            nc.vector.tensor_tensor(out=nsv[:], in0=al_ps[:], in1=w_sb[:, 15:16],
                                    op=OP.add)

            # scatters (all 112 indices valid & distinct)
            nc.gpsimd.indirect_dma_start(
                out=align_out[:, None],
                out_offset=bass.IndirectOffsetOnAxis(ap=idx[:, 2:3], axis=0),
                in_=al112[:], in_offset=None,
            )
            nc.gpsimd.indirect_dma_start(
                out=ns_out[:, None],
                out_offset=bass.IndirectOffsetOnAxis(ap=idx[:, 2:3], axis=0),
                in_=nsv[:], in_offset=None,
            )

    import bass_rust as _br
    _br.move_matmul_waits_to_ldweights(nc.m)
    _br.generate_event_semaphores(nc)
    return nc


def _get_nc():
    if "nc" not in _CACHE:
        _CACHE["nc"] = _build()
    return _CACHE["nc"]


def _prep_core(ci, query, state, memory, conv_w, conv_b, loc_w, v_a, b_a, pm):
    """Build the per-core input map (host-side index/constant prep)."""
    lo = ci * BL
    st = state[lo:lo + BL]                      # [BL, T]
    pmc = pm[lo:lo + BL].astype(np.int64)       # [BL]

    s = np.clip(pmc - 4, 0, T - WIN)            # window starts, always in-bounds
    j = np.arange(WIN)
    tp = s[:, None] + j[None, :]                # [BL, WIN] lane positions
    member = (tp >= (pmc[:, None] - 4)) & (tp <= (pmc[:, None] + 2))

    bb = np.arange(BL)[:, None]
    idx_win = (bb * TP + tp).reshape(P)         # into spad (covers t-15..t+16)
    idx_mem = (bb * T + tp).reshape(P)          # row index into mem2d
    idx_scat = (bb * T + tp).reshape(P)         # flat into [BL*T] outputs
    idx = np.stack([idx_win, idx_mem, idx_scat], axis=1).astype(np.int32)

    spad = np.zeros(SPAD_LEN, np.float32)
    sp2 = spad[:BL * TP].reshape(BL, TP)
    sp2[:, 15:15 + T] = st

    wmask = np.where(member, np.float32(0.0), NEG).reshape(1, P).astype(np.float32)
    tvd = np.where(member, tp.astype(np.float32) - BIG, np.float32(0.0))
    tvd = tvd.reshape(1, P).astype(np.float32)
    onehot = (np.arange(P)[None, :] // WIN == np.arange(BL)[:, None]).astype(np.float32)

    return {
        "mem2d": np.ascontiguousarray(memory[lo:lo + BL].reshape(BL * T, U)),
        "spad": spad,
        "state_flat": np.ascontiguousarray(st.reshape(BL * T)),
        "q_in": np.ascontiguousarray(query[lo:lo + BL]),
        "convw_in": np.ascontiguousarray(conv_w[:, 0, :]),
        "convb_in": np.ascontiguousarray(conv_b.reshape(F, 1)),
        "locw_in": np.ascontiguousarray(loc_w),
        "ba_in": np.ascontiguousarray(b_a.reshape(1, U)),
        "va_in": np.ascontiguousarray(v_a.reshape(U, 1)),
        "idx_in": idx,
        "wmask_in": wmask,
        "tvd_in": tvd,
        "oneh_in": onehot,
        "ones_in": np.ones((1, P), np.float32),
        "ident_in": np.eye(P, dtype=np.float32),
    }


def _get_runner():
    """Cached jitted SPMD executor mirroring bass2jax.run_bass_via_pjrt's
    multi-core branch (so repeat kernel() calls don't retrace)."""
    if "runner" in _CACHE:
        return _CACHE["runner"]

    import jax
    from jax.sharding import Mesh, PartitionSpec
    from jax.experimental.shard_map import shard_map
    from concourse import bass2jax, mybir

    nc = _get_nc()
    if not nc.is_finalized():
        nc.finalize()
    bass2jax.install_neuronx_cc_hook()

    partition_name = nc.partition_id_tensor.name if nc.partition_id_tensor else None
    in_names, out_names, out_avals, zero_outs = [], [], [], []
    for alloc in nc.m.functions[0].allocations:
        if not isinstance(alloc, mybir.MemoryLocationSet):
            continue
        name = alloc.memorylocations[0].name
        if alloc.kind == "ExternalInput":
            if name != partition_name:
                in_names.append(name)
        elif alloc.kind == "ExternalOutput":
            out_names.append(name)
            shape = tuple(alloc.tensor_shape)
            dtype = mybir.dt.np(alloc.dtype)
            out_avals.append(jax.core.ShapedArray(shape, dtype))
            zero_outs.append(np.zeros(shape, dtype))
    n_params = len(in_names)
    n_outs = len(out_avals)
    all_names = list(in_names) + list(out_names)
    if partition_name is not None:
        all_names.append(partition_name)

    def _body(*args):
        operands = list(args)
        if partition_name is not None:
            operands.append(bass2jax.partition_id_tensor())
        outs = bass2jax._bass_exec_p.bind(
            *operands,
            out_avals=tuple(out_avals),
            in_names=tuple(all_names),
            out_names=tuple(out_names),
            lowering_input_output_aliases=(),
            sim_require_finite=True,
            sim_require_nnan=True,
            nc=nc,
        )
        return tuple(outs)

    devices = jax.devices()[:NCORES]
    mesh = Mesh(np.asarray(devices), ("core",))
    in_specs = (PartitionSpec("core"),) * (n_params + n_outs)
    out_specs = (PartitionSpec("core"),) * n_outs
    donate = tuple(range(n_params, n_params + n_outs))
    sharded = jax.jit(
        shard_map(_body, mesh=mesh, in_specs=in_specs, out_specs=out_specs,
                  check_rep=False),
        donate_argnums=donate, keep_unused=True,
    )

    runner = {
        "sharded": sharded, "in_names": in_names, "out_names": out_names,
        "zero_outs": zero_outs, "out_avals": out_avals, "mesh": mesh,
    }
    _CACHE["runner"] = runner
    return runner


def _run(in_maps):
    r = _get_runner()
    n = NCORES
    concat_in = [
        np.concatenate([np.asarray(in_maps[c][name]) for c in range(n)], axis=0)
        for name in r["in_names"]
    ]
    concat_zeros = [np.zeros((n * z.shape[0], *z.shape[1:]), z.dtype)
                    for z in r["zero_outs"]]
    out_arrs = r["sharded"](*concat_in, *concat_zeros)
    return [
        {name: np.asarray(out_arrs[i]).reshape(n, *r["out_avals"][i].shape)[c]
         for i, name in enumerate(r["out_names"])}
        for c in range(n)
    ]


def kernel(query, state, memory, conv_w, conv_b, loc_w, v_a, b_a,
           prev_max_attentions):
    query = np.asarray(query, np.float32)
    state = np.asarray(state, np.float32)
    memory = np.asarray(memory, np.float32)
    conv_w = np.asarray(conv_w, np.float32)
    conv_b = np.asarray(conv_b, np.float32)
    loc_w = np.asarray(loc_w, np.float32)
    v_a = np.asarray(v_a, np.float32)
    b_a = np.asarray(b_a, np.float32)
    pm = np.asarray(prev_max_attentions, np.int32)

    in_maps = [
        _prep_core(ci, query, state, memory, conv_w, conv_b, loc_w, v_a, b_a, pm)
        for ci in range(NCORES)
    ]
    res = _run(in_maps)

    alignments = np.concatenate(
        [res[ci]["align_out"].reshape(BL, T) for ci in range(NCORES)], axis=0)
    next_state = np.concatenate(
        [res[ci]["ns_out"].reshape(BL, T) for ci in range(NCORES)], axis=0)
    max_att = np.concatenate(
        [res[ci]["ma_out"].reshape(BL) for ci in range(NCORES)], axis=0).astype(np.int32)
    return alignments, next_state, max_att


# revision 4
# speedup vs baseline: 2.0132x; 2.0132x over previous
"""Trainium2 Bass kernel for LocationSensitiveAttention (sparse window attention).

The reference masks every position outside a 7-wide window around
prev_max_attentions[b] with -2^32 before the softmax, so alignments are
exactly zero outside that window.  Per core (16 batch rows) the kernel:

  - block-gathers (16 descriptors) the 7 contiguous relevant memory rows
    per batch row and the conv state windows via indirect DMA,
  - restructures them through tiny DRAM bounce buffers (a DRAM round trip
    is the only way to move data across SBUF partitions),
  - computes the Bahdanau energy for the 112 = 16*7 lanes with 6 small
    matmuls (conv-as-matmul, location features, memory transpose, query &
    bias rank-1 updates, v_a contraction),
  - does a 7-lane masked softmax + argmax in [16, 7] layout,
  - block-scatters alignments into the pre-zeroed output and the
    next_state update over a bulk DRAM->DRAM copy of state.

The window start is clamped to [0, T-7] so all gather/scatter indices are
valid; lanes outside the reference's mask get energy NEG -> softmax 0,
which scatters harmlessly.

Sharding: data-parallel over batch B=128 across 8 cores (16 rows each);
the small conv/dense/v_a parameters are replicated.
"""
import sys

sys.path.insert(0, "/opt/trn_rl_repo")

import numpy as np

B, T, U, F, K = 128, 4096, 128, 32, 31
WIN = 7
NCORES = 8
BL = B // NCORES           # 16 rows per core
P = BL * WIN               # 112 lanes per core
TP = T + 32                # padded state row pitch (15 front, 17 back)
WB = 40                    # window block width (needs >= 38)
SPAD_LEN = BL * TP + 64    # tail pad so block gathers stay in-bounds
NEG = np.float32(-2.0**32 + 1)
BIG = np.float32(1048576.0)  # 2^20: t - BIG + BIG is exact in f32

_CACHE = {}


def _build():
    from concourse import bass, mybir
    import concourse.tile as tile

    f32, i32 = mybir.dt.float32, mybir.dt.int32
    AF = mybir.ActivationFunctionType
    OP = mybir.AluOpType

    def dview(ap, off, dims):
        # raw strided view of a DRAM tensor: dims = [(stride, num), ...]
        return bass.AP(ap.tensor, ap.offset + off, [[s, n] for s, n in dims])

    nc = bass.Bass()

    mem2d = nc.declare_dram_parameter("mem2d", [BL * T, U], f32, isOutput=False)
    spad = nc.declare_dram_parameter("spad", [SPAD_LEN], f32, isOutput=False)
    state_flat = nc.declare_dram_parameter("state_flat", [BL * T], f32, isOutput=False)
    q_in = nc.declare_dram_parameter("q_in", [BL, U], f32, isOutput=False)
    convw_in = nc.declare_dram_parameter("convw_in", [K, F], f32, isOutput=False)
    convb_in = nc.declare_dram_parameter("convb_in", [F, 1], f32, isOutput=False)
    locw_in = nc.declare_dram_parameter("locw_in", [F, U], f32, isOutput=False)
    ba_in = nc.declare_dram_parameter("ba_in", [1, U], f32, isOutput=False)
    va_in = nc.declare_dram_parameter("va_in", [U, 1], f32, isOutput=False)
    idx_in = nc.declare_dram_parameter("idx_in", [BL, 3], i32, isOutput=False)
    wmask_in = nc.declare_dram_parameter("wmask_in", [BL, WIN], f32, isOutput=False)
    tvd_in = nc.declare_dram_parameter("tvd_in", [BL, WIN], f32, isOutput=False)
    oneh_in = nc.declare_dram_parameter("oneh_in", [BL, P], f32, isOutput=False)
    ones_in = nc.declare_dram_parameter("ones_in", [1, P], f32, isOutput=False)
    ident_in = nc.declare_dram_parameter("ident_in", [P, P], f32, isOutput=False)

    # align_out relies on PJRT zero-donated output buffers
    align_out = nc.declare_dram_parameter("align_out", [BL * T], f32, isOutput=True)
    ns_out = nc.declare_dram_parameter("ns_out", [BL * T], f32, isOutput=True)
    ma_out = nc.declare_dram_parameter("ma_out", [BL], i32, isOutput=True)

    with tile.TileContext(nc) as tc:
        with tc.tile_pool(name="sb", bufs=1) as sb, \
             tc.tile_pool(name="ps", bufs=1, space="PSUM") as ps, \
             tc.tile_pool(name="dr", bufs=1, space="DRAM") as dr:
            # bulk next_state = state copy, issued first (no deps)
            nc.sync.dma_start(out=ns_out[:], in_=state_flat[:])

            # small loads (params off the critical path)
            idx = sb.tile([BL, 3], i32)
            nc.gpsimd.dma_start(out=idx[:], in_=idx_in[:])
            q_sb = sb.tile([BL, U], f32)
            nc.sync.dma_start(out=q_sb[:], in_=q_in[:])
            cw_sb = sb.tile([K, F], f32)
            nc.sync.dma_start(out=cw_sb[:], in_=convw_in[:])
            cb_sb = sb.tile([F, 1], f32)
            nc.sync.dma_start(out=cb_sb[:], in_=convb_in[:])
            lw_sb = sb.tile([F, U], f32)
            nc.sync.dma_start(out=lw_sb[:], in_=locw_in[:])
            ba_sb = sb.tile([1, U], f32)
            nc.sync.dma_start(out=ba_sb[:], in_=ba_in[:])
            va_sb = sb.tile([U, 1], f32)
            nc.sync.dma_start(out=va_sb[:], in_=va_in[:])
            wm_sb = sb.tile([BL, WIN], f32)
            nc.sync.dma_start(out=wm_sb[:], in_=wmask_in[:])
            tvd_sb = sb.tile([BL, WIN], f32)
            nc.sync.dma_start(out=tvd_sb[:], in_=tvd_in[:])
            oh_sb = sb.tile([BL, P], f32)
            nc.sync.dma_start(out=oh_sb[:], in_=oneh_in[:])
            on_sb = sb.tile([1, P], f32)
            nc.sync.dma_start(out=on_sb[:], in_=ones_in[:])
            id_sb = sb.tile([P, P], f32)
            nc.sync.dma_start(out=id_sb[:], in_=ident_in[:])

            # block gathers: 16 descriptors each
            wbuf = sb.tile([BL, WB], f32)   # state window block per row
            nc.gpsimd.indirect_dma_start(
                out=wbuf[:], out_offset=None, in_=spad[:, None],
                in_offset=bass.IndirectOffsetOnAxis(ap=idx[:, 0:1], axis=0))
            mbuf = sb.tile([BL, WIN * U], f32)  # 7 memory rows per batch row
            nc.gpsimd.indirect_dma_start(
                out=mbuf[:], out_offset=None, in_=mem2d[:],
                in_offset=bass.IndirectOffsetOnAxis(ap=idx[:, 1:2], axis=0))

            # bounce through DRAM to repartition
            dw = dr.tile([BL, WB], f32, space="DRAM")
            nc.sync.dma_start(out=dw[:], in_=wbuf[:])
            dm = dr.tile([BL, WIN * U], f32, space="DRAM")
            nc.sync.dma_start(out=dm[:], in_=mbuf[:])

            # Hankel read: wt[i, (b,j)] = dw[b, j+i] = state[b, t_p + i - 15]
            wt_sb = sb.tile([32, P], f32)
            nc.sync.dma_start(out=wt_sb[:],
                              in_=dview(dw[:], 0, [(1, 32), (WB, BL), (1, WIN)]))
            # state values at the lane positions: dw[b, j+15]
            sv_sb = sb.tile([BL, WIN], f32)
            nc.sync.dma_start(out=sv_sb[:],
                              in_=dview(dw[:], 15, [(WB, BL), (1, WIN)]))
            # memory rows as [112 lanes, 128]
            mem_sb = sb.tile([P, U], f32)
            nc.sync.dma_start(out=mem_sb[:],
                              in_=dview(dm[:], 0, [(U, P), (1, U)]))

            # conv: loc_feat^T [F, P] = conv_w^T @ W^T + conv_b
            lf_ps = ps.tile([F, P], f32, space="PSUM")
            nc.tensor.matmul(out=lf_ps[:], lhsT=cw_sb[:], rhs=wt_sb[0:K, :],
                             start=True, stop=True)
            lf_sb = sb.tile([F, P], f32)
            nc.scalar.activation(out=lf_sb[:], in_=lf_ps[:], func=AF.Identity,
                                 bias=cb_sb[:, 0:1])

            # big accumulation [U, P]: mem^T + loc_w^T @ lf + query + b_a
            big_ps = ps.tile([U, P], f32, space="PSUM")
            nc.tensor.matmul(out=big_ps[:], lhsT=mem_sb[:], rhs=id_sb[:],
                             start=True, stop=False)
            nc.tensor.matmul(out=big_ps[:], lhsT=lw_sb[:], rhs=lf_sb[:],
                             start=False, stop=False)
            nc.tensor.matmul(out=big_ps[:], lhsT=q_sb[:], rhs=oh_sb[:],
                             start=False, stop=False)
            nc.tensor.matmul(out=big_ps[:], lhsT=ba_sb[:], rhs=on_sb[:],
                             start=False, stop=True)
            th_sb = sb.tile([U, P], f32)
            nc.scalar.activation(out=th_sb[:], in_=big_ps[:], func=AF.Tanh)

            # energy e[0, (b,j)] = sum_u v_a[u] * tanh[u, (b,j)]
            e_ps = ps.tile([1, P], f32, space="PSUM")
            nc.tensor.matmul(out=e_ps[:], lhsT=va_sb[:], rhs=th_sb[:],
                             start=True, stop=True)
            e_sb = sb.tile([1, P], f32)
            nc.vector.tensor_copy(out=e_sb[:], in_=e_ps[:])
            # bounce to [16, 7] per-row layout
            de = dr.tile([1, P], f32, space="DRAM")
            nc.sync.dma_start(out=de[:], in_=e_sb[:])
            e16 = sb.tile([BL, WIN], f32)
            nc.sync.dma_start(out=e16[:],
                              in_=dview(de[:], 0, [(WIN, BL), (1, WIN)]))

            # masked softmax over the 7 lanes per row
            em = sb.tile([BL, WIN], f32)
            nc.vector.tensor_tensor(out=em[:], in0=e16[:], in1=wm_sb[:], op=OP.add)
            rowmax = sb.tile([BL, 1], f32)
            nc.vector.tensor_reduce(out=rowmax[:], in_=em[:],
                                    axis=mybir.AxisListType.X, op=OP.max)
            negmax = sb.tile([BL, 1], f32)
            nc.vector.tensor_scalar(out=negmax[:], in0=rowmax[:], scalar1=-1.0,
                                    scalar2=None, op0=OP.mult)
            ex = sb.tile([BL, WIN], f32)
            nc.scalar.activation(out=ex[:], in_=em[:], func=AF.Exp,
                                 bias=negmax[:, 0:1])
            rowsum = sb.tile([BL, 1], f32)
            nc.vector.tensor_reduce(out=rowsum[:], in_=ex[:],
                                    axis=mybir.AxisListType.X, op=OP.add)
            rc = sb.tile([BL, 1], f32)
            nc.vector.reciprocal(out=rc[:], in_=rowsum[:])
            al16 = sb.tile([BL, WIN], f32)
            nc.vector.tensor_scalar(out=al16[:], in0=ex[:], scalar1=rc[:, 0:1],
                                    scalar2=None, op0=OP.mult)

            # argmax: first lane with ex == 1.0 (max-subtracted exp peaks at 1)
            eq = sb.tile([BL, WIN], f32)
            nc.vector.tensor_scalar(out=eq[:], in0=ex[:], scalar1=1.0,
                                    scalar2=None, op0=OP.is_equal)
            cd = sb.tile([BL, WIN], f32)
            nc.vector.tensor_tensor(out=cd[:], in0=eq[:], in1=tvd_sb[:], op=OP.mult)
            cd2 = sb.tile([BL, WIN], f32)
            nc.vector.tensor_scalar(out=cd2[:], in0=cd[:], scalar1=float(BIG),
                                    scalar2=None, op0=OP.add)
            argt = sb.tile([BL, 1], f32)
            nc.vector.tensor_reduce(out=argt[:], in_=cd2[:],
                                    axis=mybir.AxisListType.X, op=OP.min)
            mai = sb.tile([BL, 1], i32)
            nc.vector.tensor_copy(out=mai[:], in_=argt[:])
            nc.sync.dma_start(out=ma_out[:], in_=mai[:])

            # next_state values at the window lanes
            nsv = sb.tile([BL, WIN], f32)
            nc.vector.tensor_tensor(out=nsv[:], in0=al16[:], in1=sv_sb[:], op=OP.add)

            # block scatters: 16 descriptors of 7 contiguous floats
            nc.gpsimd.indirect_dma_start(
                out=align_out[:, None],
                out_offset=bass.IndirectOffsetOnAxis(ap=idx[:, 2:3], axis=0),
                in_=al16[:], in_offset=None)
            nc.gpsimd.indirect_dma_start(
                out=ns_out[:, None],
                out_offset=bass.IndirectOffsetOnAxis(ap=idx[:, 2:3], axis=0),
                in_=nsv[:], in_offset=None)

    import bass_rust as _br
    _br.move_matmul_waits_to_ldweights(nc.m)
    _br.generate_event_semaphores(nc)
    return nc


def _get_nc():
    if "nc" not in _CACHE:
        _CACHE["nc"] = _build()
    return _CACHE["nc"]


def _prep_core(ci, query, state, memory, conv_w, conv_b, loc_w, v_a, b_a, pm):
    """Build the per-core input map (host-side index/constant prep)."""
    lo = ci * BL
    st = state[lo:lo + BL]                      # [BL, T]
    pmc = pm[lo:lo + BL].astype(np.int64)       # [BL]

    s = np.clip(pmc - 4, 0, T - WIN)            # window starts, always in-bounds
    j = np.arange(WIN)
    tp = s[:, None] + j[None, :]                # [BL, WIN] lane positions
    member = (tp >= (pmc[:, None] - 4)) & (tp <= (pmc[:, None] + 2))

    bb = np.arange(BL)
    idx_win = bb * TP + s                       # spad elem idx of window block
    idx_mem = bb * T + s                        # mem2d row idx of first window row
    idx_scat = bb * T + s                       # flat elem idx into outputs
    idx = np.stack([idx_win, idx_mem, idx_scat], axis=1).astype(np.int32)

    spad = np.zeros(SPAD_LEN, np.float32)
    sp2 = spad[:BL * TP].reshape(BL, TP)
    sp2[:, 15:15 + T] = st

    wmask = np.where(member, np.float32(0.0), NEG).astype(np.float32)
    tvd = np.where(member, tp.astype(np.float32) - BIG,
                   np.float32(0.0)).astype(np.float32)
    onehot = (np.arange(P)[None, :] // WIN == bb[:, None]).astype(np.float32)

    return {
        "mem2d": np.ascontiguousarray(memory[lo:lo + BL].reshape(BL * T, U)),
        "spad": spad,
        "state_flat": np.ascontiguousarray(st.reshape(BL * T)),
        "q_in": np.ascontiguousarray(query[lo:lo + BL]),
        "convw_in": np.ascontiguousarray(conv_w[:, 0, :]),
        "convb_in": np.ascontiguousarray(conv_b.reshape(F, 1)),
        "locw_in": np.ascontiguousarray(loc_w),
        "ba_in": np.ascontiguousarray(b_a.reshape(1, U)),
        "va_in": np.ascontiguousarray(v_a.reshape(U, 1)),
        "idx_in": idx,
        "wmask_in": wmask,
        "tvd_in": tvd,
        "oneh_in": onehot,
        "ones_in": np.ones((1, P), np.float32),
        "ident_in": np.eye(P, dtype=np.float32),
    }


def _get_runner():
    """Cached jitted SPMD executor mirroring bass2jax.run_bass_via_pjrt's
    multi-core branch (so repeat kernel() calls don't retrace)."""
    if "runner" in _CACHE:
        return _CACHE["runner"]

    import jax
    from jax.sharding import Mesh, PartitionSpec
    from jax.experimental.shard_map import shard_map
    from concourse import bass2jax, mybir

    nc = _get_nc()
    if not nc.is_finalized():
        nc.finalize()
    bass2jax.install_neuronx_cc_hook()

    partition_name = nc.partition_id_tensor.name if nc.partition_id_tensor else None
    in_names, out_names, out_avals, zero_outs = [], [], [], []
    for alloc in nc.m.functions[0].allocations:
        if not isinstance(alloc, mybir.MemoryLocationSet):
            continue
        name = alloc.memorylocations[0].name
        if alloc.kind == "ExternalInput":
            if name != partition_name:
                in_names.append(name)
        elif alloc.kind == "ExternalOutput":
            out_names.append(name)
            shape = tuple(alloc.tensor_shape)
            dtype = mybir.dt.np(alloc.dtype)
            out_avals.append(jax.core.ShapedArray(shape, dtype))
            zero_outs.append(np.zeros(shape, dtype))
    n_params = len(in_names)
    n_outs = len(out_avals)
    all_names = list(in_names) + list(out_names)
    if partition_name is not None:
        all_names.append(partition_name)

    def _body(*args):
        operands = list(args)
        if partition_name is not None:
            operands.append(bass2jax.partition_id_tensor())
        outs = bass2jax._bass_exec_p.bind(
            *operands,
            out_avals=tuple(out_avals),
            in_names=tuple(all_names),
            out_names=tuple(out_names),
            lowering_input_output_aliases=(),
            sim_require_finite=True,
            sim_require_nnan=True,
            nc=nc,
        )
        return tuple(outs)

    devices = jax.devices()[:NCORES]
    mesh = Mesh(np.asarray(devices), ("core",))
    in_specs = (PartitionSpec("core"),) * (n_params + n_outs)
    out_specs = (PartitionSpec("core"),) * n_outs
    donate = tuple(range(n_params, n_params + n_outs))
    sharded = jax.jit(
        shard_map(_body, mesh=mesh, in_specs=in_specs, out_specs=out_specs,
                  check_rep=False),
        donate_argnums=donate, keep_unused=True,
    )

    runner = {
        "sharded": sharded, "in_names": in_names, "out_names": out_names,
        "zero_outs": zero_outs, "out_avals": out_avals, "mesh": mesh,
    }
    _CACHE["runner"] = runner
    return runner


def _run(in_maps):
    r = _get_runner()
    n = NCORES
    concat_in = [
        np.concatenate([np.asarray(in_maps[c][name]) for c in range(n)], axis=0)
        for name in r["in_names"]
    ]
    concat_zeros = [np.zeros((n * z.shape[0], *z.shape[1:]), z.dtype)
                    for z in r["zero_outs"]]
    out_arrs = r["sharded"](*concat_in, *concat_zeros)
    return [
        {name: np.asarray(out_arrs[i]).reshape(n, *r["out_avals"][i].shape)[c]
         for i, name in enumerate(r["out_names"])}
        for c in range(n)
    ]


def kernel(query, state, memory, conv_w, conv_b, loc_w, v_a, b_a,
           prev_max_attentions):
    query = np.asarray(query, np.float32)
    state = np.asarray(state, np.float32)
    memory = np.asarray(memory, np.float32)
    conv_w = np.asarray(conv_w, np.float32)
    conv_b = np.asarray(conv_b, np.float32)
    loc_w = np.asarray(loc_w, np.float32)
    v_a = np.asarray(v_a, np.float32)
    b_a = np.asarray(b_a, np.float32)
    pm = np.asarray(prev_max_attentions, np.int32)

    in_maps = [
        _prep_core(ci, query, state, memory, conv_w, conv_b, loc_w, v_a, b_a, pm)
        for ci in range(NCORES)
    ]
    res = _run(in_maps)

    alignments = np.concatenate(
        [res[ci]["align_out"].reshape(BL, T) for ci in range(NCORES)], axis=0)
    next_state = np.concatenate(
        [res[ci]["ns_out"].reshape(BL, T) for ci in range(NCORES)], axis=0)
    max_att = np.concatenate(
        [res[ci]["ma_out"].reshape(BL) for ci in range(NCORES)], axis=0).astype(np.int32)
    return alignments, next_state, max_att
